# revision 7
# baseline (speedup 1.0000x reference)
"""Trainium2 Bass kernel for nn_AttentionInPnts (sparse local attention over points).

Math (per batch b, point n):
  q = wq @ xc, k_j = wk @ x_j, v_j = wv @ x_j   (x_16 == xc, the center)
  logit_j = (q . k_j) / 8 = xc^T (wq^T wk) x_j / 8 = y . x_j / 8
  a = softmax(logit)                            (17 entries)
  out = sum_j a_j v_j = wv @ (sum_j a_j x_j)

Per point-tile of 128 points:
  PE:  y[p,c'] = sum_c fcT[c,p] * A[c,c']            (1 matmul)
  DVE: t = xa * y_bc (bf16 2x), then a 2-level pairwise-add tree + a small
       tensor_reduce produce the logits (tensor_reduce alone runs at 1x, the
       tensor_tensor adds run at 2x, so the tree is cheaper).
  Act: e = exp(logit/8) with accum_out -> sum_e;  DVE: inv = 1/sum_e
  Weighted sum via the diagonal-rhs trick, diag laid out [p', j, p] so the
  matmul rhs columns are unit-stride:
    j = 0..14: one GpSimd local_scatter builds diag[:, 0:15, :]
    j = 15:    DVE tensor_scalar ident2 * e15 into a separate slice tile
    j = 16:    (center) DVE tensor_scalar s2c = xc * e16, folded into the
               PSUM accumulation via one extra matmul with rhs = ident2.
  PE:  s[c,p] = sum_j xa_j^T @ diag_j  (17 matmuls, PSUM accum)
  Act: s -> bf16, then PE: o = s^T @ wvT, Act: o * inv -> bf16 out.

Output is written bf16 and upcast to f32 on the host (halves output DMA).
Sharding: pure data-parallel, batch b -> core b (8 batches, 8 cores).
"""

import os

import numpy as np

BS = 8
NPTS = 4096
KNB = 16
C = 128
J = KNB + 1  # 16 near + 1 center
JE = J + 1  # e buffer width
JSC = 15  # j slices built by the gpsimd scatter (region 15*128 <= 2046)
P = 128  # points per tile
NTILES = NPTS // P
SCALE = 1.0 / 8.0  # 1/sqrt(c//2)

_cache = {}

# set by kernel() when tracing is enabled (BASS_KERNEL_TRACE=1)
last_exec_ns = None
last_results = None


def _build():
    import concourse.bass as bass
    import concourse.tile as tile
    from concourse import bacc, mybir

    f32 = mybir.dt.float32
    bf16 = mybir.dt.bfloat16
    i16 = mybir.dt.int16
    nc = bacc.Bacc()

    xfull = nc.declare_dram_parameter("xfull", [NPTS, J, C], bf16, isOutput=False)
    fcT = nc.declare_dram_parameter("fcT", [C, NPTS], bf16, isOutput=False)
    amat = nc.declare_dram_parameter("amat", [C, C], bf16, isOutput=False)
    wvt = nc.declare_dram_parameter("wvt", [C, C], bf16, isOutput=False)
    ident2 = nc.declare_dram_parameter("ident2", [P, P], bf16, isOutput=False)
    sidxA = nc.declare_dram_parameter("sidxA", [P, 8], i16, isOutput=False)
    sidxB = nc.declare_dram_parameter("sidxB", [P, 8], i16, isOutput=False)
    out = nc.declare_dram_parameter("out", [NPTS, C], bf16, isOutput=True)

    with tile.TileContext(nc) as tc:
        with (
            tc.tile_pool(name="consts", bufs=1) as consts,
            tc.tile_pool(name="big", bufs=4) as big,
            tc.tile_pool(name="tb", bufs=2) as tb,
            tc.tile_pool(name="diagp", bufs=3) as diagp,
            tc.tile_pool(name="small", bufs=6) as small,
            tc.tile_pool(name="psA", bufs=2, space="PSUM") as psA,
            tc.tile_pool(name="psS", bufs=3, space="PSUM") as psS,
        ):
            amat_sb = consts.tile([C, C], bf16)
            nc.sync.dma_start(out=amat_sb, in_=amat[:])
            wvt_sb = consts.tile([C, C], bf16)
            nc.sync.dma_start(out=wvt_sb, in_=wvt[:])
            ident2_sb = consts.tile([P, P], bf16)
            nc.sync.dma_start(out=ident2_sb, in_=ident2[:])
            sidxA_sb = consts.tile([P, 8], i16)
            nc.sync.dma_start(out=sidxA_sb, in_=sidxA[:])
            sidxB_sb = consts.tile([P, 8], i16)
            nc.sync.dma_start(out=sidxB_sb, in_=sidxB[:])
            fcT_sb = consts.tile([C, NPTS], bf16)
            nc.sync.dma_start(out=fcT_sb, in_=fcT[:])

            for it in range(NTILES):
                r0 = it * P
                xa = big.tile([P, J, C], bf16)
                nc.sync.dma_start(out=xa[:], in_=xfull[r0 : r0 + P, :, :])

                # y[p, c'] = sum_c xc[p, c] * A[c, c']  (fp32 accumulate)
                y_ps = psA.tile([P, C], f32)
                nc.tensor.matmul(
                    y_ps, lhsT=fcT_sb[:, r0 : r0 + P], rhs=amat_sb[:], start=True, stop=True
                )
                y_sb = small.tile([P, C], bf16)
                nc.scalar.copy(y_sb, y_ps)

                # t[p, j, c] = xa * y (y broadcast over j); bf16 2x mode
                y_ap = y_sb[:]
                y_bc = bass.AP(
                    tensor=y_ap.tensor,
                    offset=y_ap.offset,
                    ap=[y_ap.ap[0], [0, J], y_ap.ap[1]],
                )
                t = tb.tile([P, J, C], bf16)
                nc.vector.tensor_tensor(out=t[:], in0=xa[:], in1=y_bc, op=mybir.AluOpType.mult)

                # logits L[p, j] = sum_c t: 2-level pairwise tree (2x) + small reduce
                u1 = tb.tile([P, J, C // 2], bf16)
                nc.vector.tensor_tensor(
                    out=u1[:], in0=t[:, :, 0 : C // 2], in1=t[:, :, C // 2 : C],
                    op=mybir.AluOpType.add,
                )
                u2 = tb.tile([P, J, C // 4], bf16)
                nc.vector.tensor_tensor(
                    out=u2[:], in0=u1[:, :, 0 : C // 4], in1=u1[:, :, C // 4 : C // 2],
                    op=mybir.AluOpType.add,
                )
                logit = small.tile([P, J], bf16)
                with nc.allow_low_precision(reason="logit c-sum fits bf16 (fp32 internal accum)"):
                    nc.vector.tensor_reduce(
                        out=logit[:], in_=u2[:],
                        axis=mybir.AxisListType.X, op=mybir.AluOpType.add,
                    )

                # e = exp(L/8) bf16, sum_e = sum_j e in fp32
                e_sb = small.tile([P, JE], bf16)
                sum_e = small.tile([P, 1], f32)
                nc.scalar.activation(
                    out=e_sb[:, 0:J],
                    in_=logit[:],
                    func=mybir.ActivationFunctionType.Exp,
                    scale=SCALE,
                    accum_out=sum_e[:],
                )
                inv = small.tile([P, 1], f32)
                nc.vector.reciprocal(inv[:], sum_e[:])
                # f32 e[15], e[16] for the per-partition multipliers (Act, not DVE)
                ef = small.tile([P, 2], f32)
                nc.scalar.activation(
                    out=ef[:],
                    in_=logit[:, 15:17],
                    func=mybir.ActivationFunctionType.Exp,
                    scale=SCALE,
                )

                # diag_j[p', p] = (p' == p) * e[p', j], j innermost-unit-stride;
                # two slabs so the first matmuls can start before slab B lands
                diagA = diagp.tile([P, 8, P], bf16)
                nc.gpsimd.local_scatter(
                    out_ap=diagA[:],
                    data_ap=e_sb[:, 0:8],
                    idxs_ap=sidxA_sb[:],
                    channels=P,
                    num_elems=8 * P,
                    num_idxs=8,
                )
                diagB = diagp.tile([P, 7, P], bf16)
                nc.gpsimd.local_scatter(
                    out_ap=diagB[:],
                    data_ap=e_sb[:, 8:16],
                    idxs_ap=sidxB_sb[:],
                    channels=P,
                    num_elems=7 * P,
                    num_idxs=8,
                )
                diag15 = small.tile([P, P], bf16)
                nc.scalar.mul(diag15[:], ident2_sb[:], ef[:, 0:1])
                # center: s2c[p, c] = e16[p] * xc[p, c]; folded in via rhs=ident2
                s2c = small.tile([P, C], bf16)
                nc.gpsimd.tensor_scalar(
                    out=s2c[:], in0=xa[:, KNB, :],
                    scalar1=ef[:, 1:2], scalar2=None,
                    op0=mybir.AluOpType.mult,
                )

                # s[c, p] = sum_j xa_j^T @ diag_j  (PSUM accumulate)
                s_ps = psS.tile([C, P], f32)
                nc.tensor.matmul(
                    s_ps, lhsT=s2c[:], rhs=ident2_sb[:], start=True, stop=False
                )
                nc.tensor.matmul(
                    s_ps, lhsT=xa[:, 15, :], rhs=diag15[:], start=False, stop=False
                )
                for j in range(8):
                    nc.tensor.matmul(
                        s_ps, lhsT=xa[:, j, :], rhs=diagA[:, j, :],
                        start=False, stop=False,
                    )
                for j in range(7):
                    nc.tensor.matmul(
                        s_ps, lhsT=xa[:, 8 + j, :], rhs=diagB[:, j, :],
                        start=False, stop=(j == 6),
                    )
                s_sb = small.tile([C, P], bf16)
                nc.scalar.copy(s_sb, s_ps)

                # o[p, c'] = (sum_c s[c, p] * wvT[c, c']) / sum_e[p]
                o_ps = psA.tile([P, C], f32)
                nc.tensor.matmul(o_ps, lhsT=s_sb[:], rhs=wvt_sb[:], start=True, stop=True)
                o_sb = small.tile([P, C], bf16)
                nc.scalar.mul(o_sb, o_ps, inv[:])

                nc.sync.dma_start(out=out[r0 : r0 + P, :], in_=o_sb[:])

    nc.compile()
    return nc


def _get_nc():
    if "nc" not in _cache:
        _cache["nc"] = _build()
    return _cache["nc"]


def _host_prep(fea_center, fea_near, wq, wk, wv):
    import ml_dtypes

    bf = ml_dtypes.bfloat16
    fea_center = np.asarray(fea_center, dtype=np.float32)
    fea_near = np.asarray(fea_near, dtype=np.float32)
    wq = np.asarray(wq, dtype=np.float32)
    wk = np.asarray(wk, dtype=np.float32)
    wv = np.asarray(wv, dtype=np.float32)

    amat = np.ascontiguousarray(wq.T @ wk).astype(bf)  # [c_center, c_near]
    wvt = np.ascontiguousarray(wv.T).astype(bf)  # [c_in, c_out]

    # [bs, n, 17, c]: near neighbors then the center as the 17th entry
    xfull = np.concatenate([fea_near, fea_center], axis=2).astype(bf)
    # transposed center features [bs, c, n]
    fcT = np.ascontiguousarray(np.transpose(fea_center[:, :, 0, :], (0, 2, 1))).astype(bf)

    ident2 = np.eye(P, dtype=np.float32).astype(bf)

    # local_scatter index tables: partition p scatters e[p, j] to j*P + p
    pp = np.arange(P, dtype=np.int16)[:, None]
    jj8 = np.arange(8, dtype=np.int16)[None, :]
    sidxA = np.ascontiguousarray(jj8 * P + pp)  # j = 0..7
    sidxB = np.full((P, 8), -1, dtype=np.int16)
    sidxB[:, 0:7] = np.arange(7, dtype=np.int16)[None, :] * P + pp  # j = 8..14

    return xfull, fcT, amat, wvt, ident2, sidxA, sidxB


def kernel(fea_center, fea_near, wq, wk, wv):
    global last_exec_ns, last_results

    from concourse.bass_utils import run_bass_kernel_spmd

    xfull, fcT, amat, wvt, ident2, sidxA, sidxB = _host_prep(fea_center, fea_near, wq, wk, wv)

    nc = _get_nc()
    in_maps = []
    for b in range(BS):
        in_maps.append(
            {
                "xfull": np.ascontiguousarray(xfull[b]),
                "fcT": np.ascontiguousarray(fcT[b]),
                "amat": amat,
                "wvt": wvt,
                "ident2": ident2,
                "sidxA": sidxA,
                "sidxB": sidxB,
            }
        )

    trace = bool(int(os.environ.get("BASS_KERNEL_TRACE", "0")))
    res = run_bass_kernel_spmd(nc, in_maps, core_ids=list(range(BS)), trace=trace)
    last_exec_ns = res.exec_time_ns
    last_results = res
    out = np.stack([res.results[b]["out"] for b in range(BS)], axis=0).astype(np.float32)
    return out


# revision 8
# speedup vs baseline: 1.2477x; 1.2477x over previous
"""Trainium2 Bass kernel for nn_AttentionInPnts (sparse local attention over points).

Math (per batch b, point n):
  q = wq @ xc, k_j = wk @ x_j, v_j = wv @ x_j   (x_16 == xc, the center)
  logit_j = (q . k_j) / 8 = xc^T (wq^T wk) x_j / 8 = y . x_j / 8
  a = softmax(logit)                            (17 entries)
  out = sum_j a_j v_j = wv @ (sum_j a_j x_j)

Per point-tile of 128 points:
  PE:  y[p,c'] = sum_c fcT[c,p] * A[c,c']            (1 matmul)
  DVE: t = xa * y_bc (bf16 2x), then a 2-level pairwise-add tree + a small
       tensor_reduce produce the logits (tensor_reduce alone runs at 1x, the
       tensor_tensor adds run at 2x, so the tree is cheaper).
  Act: e = exp(logit/8) with accum_out -> sum_e;  DVE: inv = 1/sum_e
  Weighted sum via the diagonal-rhs trick, diag laid out [p', j, p] so the
  matmul rhs columns are unit-stride:
    j = 0..14: one GpSimd local_scatter builds diag[:, 0:15, :]
    j = 15:    DVE tensor_scalar ident2 * e15 into a separate slice tile
    j = 16:    (center) DVE tensor_scalar s2c = xc * e16, folded into the
               PSUM accumulation via one extra matmul with rhs = ident2.
  PE:  s[c,p] = sum_j xa_j^T @ diag_j  (17 matmuls, PSUM accum)
  Act: s -> bf16, then PE: o = s^T @ wvT, Act: o * inv -> bf16 out.

Output is written bf16 and upcast to f32 on the host (halves output DMA).
Sharding: pure data-parallel, batch b -> core b (8 batches, 8 cores).
"""

import os

import numpy as np

BS = 8
NPTS = 4096
KNB = 16
C = 128
J = KNB + 1  # 16 near + 1 center
JE = J + 1  # e buffer width
JSC = 15  # j slices built by the gpsimd scatter (region 15*128 <= 2046)
P = 128  # points per tile
NTILES = NPTS // P
SCALE = 1.0 / 8.0  # 1/sqrt(c//2)

_cache = {}

# set by kernel() when tracing is enabled (BASS_KERNEL_TRACE=1)
last_exec_ns = None
last_results = None


def _build():
    import concourse.bass as bass
    import concourse.tile as tile
    from concourse import bacc, mybir

    f32 = mybir.dt.float32
    bf16 = mybir.dt.bfloat16
    i16 = mybir.dt.int16
    nc = bacc.Bacc()

    xfull = nc.declare_dram_parameter("xfull", [NPTS, J, C], bf16, isOutput=False)
    fcT = nc.declare_dram_parameter("fcT", [C, NPTS], bf16, isOutput=False)
    amat = nc.declare_dram_parameter("amat", [C, C], bf16, isOutput=False)
    wvt = nc.declare_dram_parameter("wvt", [C, C], bf16, isOutput=False)
    sidxA = nc.declare_dram_parameter("sidxA", [P, 8], i16, isOutput=False)
    sidxB = nc.declare_dram_parameter("sidxB", [P, 10], i16, isOutput=False)
    out = nc.declare_dram_parameter("out", [NPTS, C], bf16, isOutput=True)

    with tile.TileContext(nc) as tc:
        with (
            tc.tile_pool(name="consts", bufs=1) as consts,
            tc.tile_pool(name="big", bufs=4) as big,
            tc.tile_pool(name="tb", bufs=2) as tb,
            tc.tile_pool(name="diagp", bufs=3) as diagp,
            tc.tile_pool(name="small", bufs=6) as small,
            tc.tile_pool(name="psA", bufs=2, space="PSUM") as psA,
            tc.tile_pool(name="psS", bufs=3, space="PSUM") as psS,
        ):
            amat_sb = consts.tile([C, C], bf16)
            nc.sync.dma_start(out=amat_sb, in_=amat[:])
            wvt_sb = consts.tile([C, C], bf16)
            nc.sync.dma_start(out=wvt_sb, in_=wvt[:])
            sidxA_sb = consts.tile([P, 8], i16)
            nc.sync.dma_start(out=sidxA_sb, in_=sidxA[:])
            sidxB_sb = consts.tile([P, 10], i16)
            nc.sync.dma_start(out=sidxB_sb, in_=sidxB[:])
            fcT_sb = consts.tile([C, NPTS], bf16)
            nc.sync.dma_start(out=fcT_sb, in_=fcT[:])

            for it in range(NTILES):
                r0 = it * P
                xa = big.tile([P, J, C], bf16)
                nc.sync.dma_start(out=xa[:], in_=xfull[r0 : r0 + P, :, :])

                # y[p, c'] = sum_c xc[p, c] * A[c, c']  (fp32 accumulate)
                y_ps = psA.tile([P, C], f32)
                nc.tensor.matmul(
                    y_ps, lhsT=fcT_sb[:, r0 : r0 + P], rhs=amat_sb[:], start=True, stop=True
                )
                y_sb = small.tile([P, C], bf16)
                nc.scalar.copy(y_sb, y_ps)

                # t[p, j, c] = xa * y (y broadcast over j); bf16 2x mode
                y_ap = y_sb[:]
                y_bc = bass.AP(
                    tensor=y_ap.tensor,
                    offset=y_ap.offset,
                    ap=[y_ap.ap[0], [0, J], y_ap.ap[1]],
                )
                t = tb.tile([P, J, C], bf16)
                nc.vector.tensor_tensor(out=t[:], in0=xa[:], in1=y_bc, op=mybir.AluOpType.mult)

                # logits L[p, j] = sum_c t: 2-level pairwise tree (2x) + small reduce
                u1 = tb.tile([P, J, C // 2], bf16)
                nc.vector.tensor_tensor(
                    out=u1[:], in0=t[:, :, 0 : C // 2], in1=t[:, :, C // 2 : C],
                    op=mybir.AluOpType.add,
                )
                u2 = tb.tile([P, J, C // 4], bf16)
                nc.vector.tensor_tensor(
                    out=u2[:], in0=u1[:, :, 0 : C // 4], in1=u1[:, :, C // 4 : C // 2],
                    op=mybir.AluOpType.add,
                )
                logit = small.tile([P, J], bf16)
                with nc.allow_low_precision(reason="logit c-sum fits bf16 (fp32 internal accum)"):
                    nc.vector.tensor_reduce(
                        out=logit[:], in_=u2[:],
                        axis=mybir.AxisListType.X, op=mybir.AluOpType.add,
                    )

                # e = exp(L/8) bf16, sum_e = sum_j e in fp32
                e_sb = small.tile([P, JE], bf16)
                sum_e = small.tile([P, 1], f32)
                nc.scalar.activation(
                    out=e_sb[:, 0:J],
                    in_=logit[:],
                    func=mybir.ActivationFunctionType.Exp,
                    scale=SCALE,
                    accum_out=sum_e[:],
                )
                inv = small.tile([P, 1], f32)
                nc.vector.reciprocal(inv[:], sum_e[:])

                # diag_j[p', p] = (p' == p) * e[p', j], j innermost-unit-stride;
                # two slabs so the first matmuls can start before slab B lands
                diagA = diagp.tile([P, 8, P], bf16)
                nc.gpsimd.local_scatter(
                    out_ap=diagA[:],
                    data_ap=e_sb[:, 0:8],
                    idxs_ap=sidxA_sb[:],
                    channels=P,
                    num_elems=8 * P,
                    num_idxs=8,
                )
                diagB = diagp.tile([P, 9, P], bf16)
                nc.gpsimd.local_scatter(
                    out_ap=diagB[:],
                    data_ap=e_sb[:, 8:JE],
                    idxs_ap=sidxB_sb[:],
                    channels=P,
                    num_elems=9 * P,
                    num_idxs=10,
                )
                # s[c, p] = sum_j xa_j^T @ diag_j  (PSUM accumulate)
                s_ps = psS.tile([C, P], f32)
                for j in range(8):
                    nc.tensor.matmul(
                        s_ps, lhsT=xa[:, j, :], rhs=diagA[:, j, :],
                        start=(j == 0), stop=False,
                    )
                for j in range(9):
                    nc.tensor.matmul(
                        s_ps, lhsT=xa[:, 8 + j, :], rhs=diagB[:, j, :],
                        start=False, stop=(j == 8),
                    )
                s_sb = small.tile([C, P], bf16)
                nc.scalar.copy(s_sb, s_ps)

                # o[p, c'] = (sum_c s[c, p] * wvT[c, c']) / sum_e[p]
                o_ps = psA.tile([P, C], f32)
                nc.tensor.matmul(o_ps, lhsT=s_sb[:], rhs=wvt_sb[:], start=True, stop=True)
                o_sb = small.tile([P, C], bf16)
                nc.scalar.mul(o_sb, o_ps, inv[:])

                nc.sync.dma_start(out=out[r0 : r0 + P, :], in_=o_sb[:])

    nc.compile()
    return nc


def _get_nc():
    if "nc" not in _cache:
        _cache["nc"] = _build()
    return _cache["nc"]


def _host_prep(fea_center, fea_near, wq, wk, wv):
    import ml_dtypes

    bf = ml_dtypes.bfloat16
    fea_center = np.asarray(fea_center, dtype=np.float32)
    fea_near = np.asarray(fea_near, dtype=np.float32)
    wq = np.asarray(wq, dtype=np.float32)
    wk = np.asarray(wk, dtype=np.float32)
    wv = np.asarray(wv, dtype=np.float32)

    amat = np.ascontiguousarray(wq.T @ wk).astype(bf)  # [c_center, c_near]
    wvt = np.ascontiguousarray(wv.T).astype(bf)  # [c_in, c_out]

    # [bs, n, 17, c]: near neighbors then the center as the 17th entry
    xfull = np.concatenate([fea_near, fea_center], axis=2).astype(bf)
    # transposed center features [bs, c, n]
    fcT = np.ascontiguousarray(np.transpose(fea_center[:, :, 0, :], (0, 2, 1))).astype(bf)

    # local_scatter index tables: partition p scatters e[p, j] to j*P + p
    pp = np.arange(P, dtype=np.int16)[:, None]
    jj8 = np.arange(8, dtype=np.int16)[None, :]
    sidxA = np.ascontiguousarray(jj8 * P + pp)  # j = 0..7
    sidxB = np.full((P, 10), -1, dtype=np.int16)
    sidxB[:, 0:9] = np.arange(9, dtype=np.int16)[None, :] * P + pp  # j = 8..16

    return xfull, fcT, amat, wvt, sidxA, sidxB


def kernel(fea_center, fea_near, wq, wk, wv):
    global last_exec_ns, last_results

    from concourse.bass_utils import run_bass_kernel_spmd

    xfull, fcT, amat, wvt, sidxA, sidxB = _host_prep(fea_center, fea_near, wq, wk, wv)

    nc = _get_nc()
    in_maps = []
    for b in range(BS):
        in_maps.append(
            {
                "xfull": np.ascontiguousarray(xfull[b]),
                "fcT": np.ascontiguousarray(fcT[b]),
                "amat": amat,
                "wvt": wvt,
                "sidxA": sidxA,
                "sidxB": sidxB,
            }
        )

    trace = bool(int(os.environ.get("BASS_KERNEL_TRACE", "0")))
    res = run_bass_kernel_spmd(nc, in_maps, core_ids=list(range(BS)), trace=trace)
    last_exec_ns = res.exec_time_ns
    last_results = res
    out = np.stack([res.results[b]["out"] for b in range(BS)], axis=0).astype(np.float32)
    return out


# revision 9
# speedup vs baseline: 1.3787x; 1.1050x over previous
"""Trainium2 Bass kernel for nn_AttentionInPnts (sparse local attention over points).

Math (per batch b, point n):
  q = wq @ xc, k_j = wk @ x_j, v_j = wv @ x_j   (x_16 == xc, the center)
  logit_j = (q . k_j) / 8 = xc^T (wq^T wk) x_j / 8 = y . x_j / 8
  a = softmax(logit)                            (17 entries)
  out = sum_j a_j v_j = wv @ (sum_j a_j x_j)

Per point-tile of 128 points:
  PE:  y[p,c'] = sum_c fcT[c,p] * A[c,c']            (1 matmul)
  DVE: t = xa * y_bc (bf16 2x), then a 2-level pairwise-add tree + a small
       tensor_reduce produce the logits (tensor_reduce alone runs at 1x, the
       tensor_tensor adds run at 2x, so the tree is cheaper).
  Act: e = exp(logit/8) with accum_out -> sum_e;  DVE: inv = 1/sum_e
  Weighted sum via the diagonal-rhs trick, diag laid out [p', j, p] so the
  matmul rhs columns are unit-stride:
    j = 0..14: one GpSimd local_scatter builds diag[:, 0:15, :]
    j = 15:    DVE tensor_scalar ident2 * e15 into a separate slice tile
    j = 16:    (center) DVE tensor_scalar s2c = xc * e16, folded into the
               PSUM accumulation via one extra matmul with rhs = ident2.
  PE:  s[c,p] = sum_j xa_j^T @ diag_j  (17 matmuls, PSUM accum)
  Act: s -> bf16, then PE: o = s^T @ wvT, Act: o * inv -> bf16 out.

Output is written bf16 and upcast to f32 on the host (halves output DMA).
Sharding: pure data-parallel, batch b -> core b (8 batches, 8 cores).
"""

import os

import numpy as np

BS = 8
NPTS = 4096
KNB = 16
C = 128
J = KNB + 1  # 16 near + 1 center
JE = J + 1  # e buffer width
JSC = 15  # j slices built by the gpsimd scatter (region 15*128 <= 2046)
P = 128  # points per tile
NTILES = NPTS // P
SCALE = 1.0 / 8.0  # 1/sqrt(c//2)

_cache = {}

# set by kernel() when tracing is enabled (BASS_KERNEL_TRACE=1)
last_exec_ns = None
last_results = None


def _build():
    import concourse.bass as bass
    import concourse.tile as tile
    from concourse import bacc, mybir

    f32 = mybir.dt.float32
    bf16 = mybir.dt.bfloat16
    i16 = mybir.dt.int16
    nc = bacc.Bacc()

    xfull = nc.declare_dram_parameter("xfull", [NPTS, J, C], bf16, isOutput=False)
    fcT = nc.declare_dram_parameter("fcT", [C, NPTS], bf16, isOutput=False)
    amat = nc.declare_dram_parameter("amat", [C, C], bf16, isOutput=False)
    wvt = nc.declare_dram_parameter("wvt", [C, C], bf16, isOutput=False)
    ident2 = nc.declare_dram_parameter("ident2", [P, P], bf16, isOutput=False)
    sidx = nc.declare_dram_parameter("sidx", [P, 16], i16, isOutput=False)
    out = nc.declare_dram_parameter("out", [NPTS, C], bf16, isOutput=True)

    with tile.TileContext(nc) as tc:
        with (
            tc.tile_pool(name="consts", bufs=1) as consts,
            tc.tile_pool(name="big", bufs=6) as big,
            tc.tile_pool(name="tb", bufs=3) as tb,
            tc.tile_pool(name="diagp", bufs=3) as diagp,
            tc.tile_pool(name="small", bufs=6) as small,
            tc.tile_pool(name="psA", bufs=2, space="PSUM") as psA,
            tc.tile_pool(name="psS", bufs=3, space="PSUM") as psS,
        ):
            amat_sb = consts.tile([C, C], bf16)
            nc.sync.dma_start(out=amat_sb, in_=amat[:])
            wvt_sb = consts.tile([C, C], bf16)
            nc.sync.dma_start(out=wvt_sb, in_=wvt[:])
            ident2_sb = consts.tile([P, P], bf16)
            nc.sync.dma_start(out=ident2_sb, in_=ident2[:])
            sidx_sb = consts.tile([P, 16], i16)
            nc.sync.dma_start(out=sidx_sb, in_=sidx[:])
            fcT_sb = consts.tile([C, NPTS], bf16)
            nc.sync.dma_start(out=fcT_sb, in_=fcT[:])

            for it in range(NTILES):
                r0 = it * P
                xa = big.tile([P, J, C], bf16)
                nc.sync.dma_start(out=xa[:], in_=xfull[r0 : r0 + P, :, :])

                # y[p, c'] = sum_c xc[p, c] * A[c, c']  (fp32 accumulate)
                y_ps = psA.tile([P, C], f32)
                nc.tensor.matmul(
                    y_ps, lhsT=fcT_sb[:, r0 : r0 + P], rhs=amat_sb[:], start=True, stop=True
                )
                y_sb = small.tile([P, C], bf16)
                nc.scalar.copy(y_sb, y_ps)

                # t[p, j, c] = xa * y (y broadcast over j); bf16 2x mode
                y_ap = y_sb[:]
                y_bc = bass.AP(
                    tensor=y_ap.tensor,
                    offset=y_ap.offset,
                    ap=[y_ap.ap[0], [0, J], y_ap.ap[1]],
                )
                t = tb.tile([P, J, C], bf16)
                nc.vector.tensor_tensor(out=t[:], in0=xa[:], in1=y_bc, op=mybir.AluOpType.mult)

                # logits L[p, j] = sum_c t: 2-level pairwise tree (2x) + small reduce
                u1 = tb.tile([P, J, C // 2], bf16)
                nc.vector.tensor_tensor(
                    out=u1[:], in0=t[:, :, 0 : C // 2], in1=t[:, :, C // 2 : C],
                    op=mybir.AluOpType.add,
                )
                u2 = tb.tile([P, J, C // 4], bf16)
                nc.vector.tensor_tensor(
                    out=u2[:], in0=u1[:, :, 0 : C // 4], in1=u1[:, :, C // 4 : C // 2],
                    op=mybir.AluOpType.add,
                )
                logit = small.tile([P, J], bf16)
                with nc.allow_low_precision(reason="logit c-sum fits bf16 (fp32 internal accum)"):
                    nc.vector.tensor_reduce(
                        out=logit[:], in_=u2[:],
                        axis=mybir.AxisListType.X, op=mybir.AluOpType.add,
                    )

                # e = exp(L/8) bf16, sum_e = sum_j e in fp32
                e_sb = small.tile([P, JE], bf16)
                sum_e = small.tile([P, 1], f32)
                nc.scalar.activation(
                    out=e_sb[:, 0:J],
                    in_=logit[:],
                    func=mybir.ActivationFunctionType.Exp,
                    scale=SCALE,
                    accum_out=sum_e[:],
                )
                inv = small.tile([P, 1], f32)
                nc.vector.reciprocal(inv[:], sum_e[:])

                # f32 e[15], e[16] for the per-partition multipliers
                ef = small.tile([P, 2], f32)
                nc.vector.tensor_copy(ef[:], e_sb[:, 15:17])

                # diag_j[p', p] = (p' == p) * e[p', j], j innermost-unit-stride
                diag = diagp.tile([P, JSC, P], bf16)
                nc.gpsimd.local_scatter(
                    out_ap=diag[:],
                    data_ap=e_sb[:, 0:16],
                    idxs_ap=sidx_sb[:],
                    channels=P,
                    num_elems=JSC * P,
                    num_idxs=16,
                )
                # j = 15 slice on the Act engine: ident2 * e15
                diag15 = small.tile([P, P], bf16)
                nc.scalar.mul(diag15[:], ident2_sb[:], ef[:, 0:1])
                # center (j = 16): s2c = e16 * xc, folded in via rhs = ident2
                s2c = small.tile([P, C], bf16)
                nc.vector.tensor_scalar(
                    out=s2c[:], in0=xa[:, KNB, :],
                    scalar1=ef[:, 1:2], scalar2=None,
                    op0=mybir.AluOpType.mult,
                )
                # s[c, p] = sum_j xa_j^T @ diag_j  (PSUM accumulate)
                s_ps = psS.tile([C, P], f32)
                nc.tensor.matmul(
                    s_ps, lhsT=s2c[:], rhs=ident2_sb[:], start=True, stop=False
                )
                nc.tensor.matmul(
                    s_ps, lhsT=xa[:, 15, :], rhs=diag15[:], start=False, stop=False
                )
                for j in range(JSC):
                    nc.tensor.matmul(
                        s_ps, lhsT=xa[:, j, :], rhs=diag[:, j, :],
                        start=False, stop=(j == JSC - 1),
                    )
                s_sb = small.tile([C, P], bf16)
                nc.scalar.copy(s_sb, s_ps)

                # o[p, c'] = (sum_c s[c, p] * wvT[c, c']) / sum_e[p]
                o_ps = psA.tile([P, C], f32)
                nc.tensor.matmul(o_ps, lhsT=s_sb[:], rhs=wvt_sb[:], start=True, stop=True)
                o_sb = small.tile([P, C], bf16)
                nc.scalar.mul(o_sb, o_ps, inv[:])

                nc.gpsimd.dma_start(out=out[r0 : r0 + P, :], in_=o_sb[:])

    nc.compile()
    return nc


def _get_nc():
    if "nc" not in _cache:
        _cache["nc"] = _build()
    return _cache["nc"]


def _host_prep(fea_center, fea_near, wq, wk, wv):
    import ml_dtypes

    bf = ml_dtypes.bfloat16
    fea_center = np.asarray(fea_center, dtype=np.float32)
    fea_near = np.asarray(fea_near, dtype=np.float32)
    wq = np.asarray(wq, dtype=np.float32)
    wk = np.asarray(wk, dtype=np.float32)
    wv = np.asarray(wv, dtype=np.float32)

    amat = np.ascontiguousarray(wq.T @ wk).astype(bf)  # [c_center, c_near]
    wvt = np.ascontiguousarray(wv.T).astype(bf)  # [c_in, c_out]

    # [bs, n, 17, c]: near neighbors then the center as the 17th entry
    xfull = np.concatenate([fea_near, fea_center], axis=2).astype(bf)
    # transposed center features [bs, c, n]
    fcT = np.ascontiguousarray(np.transpose(fea_center[:, :, 0, :], (0, 2, 1))).astype(bf)

    ident2 = np.eye(P, dtype=np.float32).astype(bf)

    # local_scatter index table: partition p scatters e[p, j] to j*P + p
    pp = np.arange(P, dtype=np.int16)[:, None]
    sidx = np.full((P, 16), -1, dtype=np.int16)
    sidx[:, 0:JSC] = np.arange(JSC, dtype=np.int16)[None, :] * P + pp  # j = 0..14

    return xfull, fcT, amat, wvt, ident2, sidx


def kernel(fea_center, fea_near, wq, wk, wv):
    global last_exec_ns, last_results

    from concourse.bass_utils import run_bass_kernel_spmd

    xfull, fcT, amat, wvt, ident2, sidx = _host_prep(fea_center, fea_near, wq, wk, wv)

    nc = _get_nc()
    in_maps = []
    for b in range(BS):
        in_maps.append(
            {
                "xfull": np.ascontiguousarray(xfull[b]),
                "fcT": np.ascontiguousarray(fcT[b]),
                "amat": amat,
                "wvt": wvt,
                "ident2": ident2,
                "sidx": sidx,
            }
        )

    trace = bool(int(os.environ.get("BASS_KERNEL_TRACE", "0")))
    res = run_bass_kernel_spmd(nc, in_maps, core_ids=list(range(BS)), trace=trace)
    last_exec_ns = res.exec_time_ns
    last_results = res
    out = np.stack([res.results[b]["out"] for b in range(BS)], axis=0).astype(np.float32)
    return out


# revision 10
# speedup vs baseline: 1.5079x; 1.0937x over previous
"""Trainium2 Bass kernel for nn_AttentionInPnts (sparse local attention over points).

Math (per batch b, point n):
  q = wq @ xc, k_j = wk @ x_j, v_j = wv @ x_j   (x_16 == xc, the center)
  logit_j = (q . k_j) / 8 = xc^T (wq^T wk) x_j / 8 = y . x_j / 8
  a = softmax(logit)                            (17 entries)
  out = sum_j a_j v_j = wv @ (sum_j a_j x_j)

Per point-tile of 128 points:
  PE:  y[p,c'] = sum_c fcT[c,p] * A[c,c']            (1 matmul)
  DVE: t = xa * y_bc (bf16 2x), then a 2-level pairwise-add tree + a small
       tensor_reduce produce the logits (tensor_reduce alone runs at 1x, the
       tensor_tensor adds run at 2x, so the tree is cheaper).
  Act: e = exp(logit/8) with accum_out -> sum_e;  DVE: inv = 1/sum_e
  Weighted sum via the diagonal-rhs trick, diag laid out [p', j, p] so the
  matmul rhs columns are unit-stride:
    j = 0..14: one GpSimd local_scatter builds diag[:, 0:15, :]
    j = 15:    DVE tensor_scalar ident2 * e15 into a separate slice tile
    j = 16:    (center) DVE tensor_scalar s2c = xc * e16, folded into the
               PSUM accumulation via one extra matmul with rhs = ident2.
  PE:  s[c,p] = sum_j xa_j^T @ diag_j  (17 matmuls, PSUM accum)
  Act: s -> bf16, then PE: o = s^T @ wvT, Act: o * inv -> bf16 out.

Output is written bf16 and upcast to f32 on the host (halves output DMA).
Sharding: pure data-parallel, batch b -> core b (8 batches, 8 cores).
"""

import os

import numpy as np

BS = 8
NPTS = 4096
KNB = 16
C = 128
J = KNB + 1  # 16 near + 1 center
JE = J + 1  # e buffer width
JSC = 15  # j slices built by the gpsimd scatter (region 15*128 <= 2046)
P = 128  # points per tile
NTILES = NPTS // P
SCALE = 1.0 / 8.0  # 1/sqrt(c//2)

_cache = {}

# set by kernel() when tracing is enabled (BASS_KERNEL_TRACE=1)
last_exec_ns = None
last_results = None


def _build():
    import concourse.bass as bass
    import concourse.tile as tile
    from concourse import bacc, mybir

    f32 = mybir.dt.float32
    bf16 = mybir.dt.bfloat16
    i16 = mybir.dt.int16
    nc = bacc.Bacc()

    xfull = nc.declare_dram_parameter("xfull", [NPTS, J, C], bf16, isOutput=False)
    fcT = nc.declare_dram_parameter("fcT", [C, NPTS], bf16, isOutput=False)
    amat = nc.declare_dram_parameter("amat", [C, C], bf16, isOutput=False)
    wvt = nc.declare_dram_parameter("wvt", [C, C], bf16, isOutput=False)
    ident2 = nc.declare_dram_parameter("ident2", [P, P], bf16, isOutput=False)
    sidx = nc.declare_dram_parameter("sidx", [P, 16], i16, isOutput=False)
    out = nc.declare_dram_parameter("out", [NPTS, C], bf16, isOutput=True)

    with tile.TileContext(nc) as tc:
        with (
            tc.tile_pool(name="consts", bufs=1) as consts,
            tc.tile_pool(name="big", bufs=6) as big,
            tc.tile_pool(name="tb", bufs=3) as tb,
            tc.tile_pool(name="diagp", bufs=3) as diagp,
            tc.tile_pool(name="small", bufs=6) as small,
            tc.tile_pool(name="psA", bufs=2, space="PSUM") as psA,
            tc.tile_pool(name="psS", bufs=3, space="PSUM") as psS,
        ):
            amat_sb = consts.tile([C, C], bf16)
            nc.sync.dma_start(out=amat_sb, in_=amat[:])
            wvt_sb = consts.tile([C, C], bf16)
            nc.sync.dma_start(out=wvt_sb, in_=wvt[:])
            ident2_sb = consts.tile([P, P], bf16)
            nc.sync.dma_start(out=ident2_sb, in_=ident2[:])
            sidx_sb = consts.tile([P, 16], i16)
            nc.sync.dma_start(out=sidx_sb, in_=sidx[:])
            fcT_sb = consts.tile([C, NPTS], bf16)
            nc.sync.dma_start(out=fcT_sb, in_=fcT[:])

            # 3-stage software pipeline: for tile i this iteration emits the
            # logits head (DMA/y/t/tree/exp/scatter) of tile i, the weighted-sum
            # matmul block of tile i-1, and the output tail of tile i-2 -- so no
            # engine queue ever holds an instruction whose dependency is more
            # than one stage away (FIFO head-of-line stalls killed the naive
            # per-tile ordering).
            st = {}
            for it in range(NTILES + 2):
                if it < NTILES:
                    r0 = it * P
                    xa = big.tile([P, J, C], bf16)
                    nc.sync.dma_start(out=xa[:], in_=xfull[r0 : r0 + P, :, :])
                    y_ps = psA.tile([P, C], f32)
                    nc.tensor.matmul(
                        y_ps, lhsT=fcT_sb[:, r0 : r0 + P], rhs=amat_sb[:],
                        start=True, stop=True,
                    )
                    y_sb = small.tile([P, C], bf16)
                    nc.scalar.copy(y_sb, y_ps)

                    # t[p, j, c] = xa * y (y broadcast over j); bf16 2x mode
                    y_ap = y_sb[:]
                    y_bc = bass.AP(
                        tensor=y_ap.tensor,
                        offset=y_ap.offset,
                        ap=[y_ap.ap[0], [0, J], y_ap.ap[1]],
                    )
                    t = tb.tile([P, J, C], bf16)
                    nc.vector.tensor_tensor(
                        out=t[:], in0=xa[:], in1=y_bc, op=mybir.AluOpType.mult
                    )
                    st[it] = {"xa": xa, "t": t}

                if it - 1 in st:
                    # DVE tail + Act diag slice of tile i-1 (slots in while
                    # tile i's tree runs; all deps long satisfied)
                    p1 = st[it - 1]
                    inv = small.tile([P, 1], f32)
                    nc.vector.reciprocal(inv[:], p1["sum_e"][:])
                    ef = small.tile([P, 2], f32)
                    nc.vector.tensor_copy(ef[:], p1["e_sb"][:, 15:17])
                    s2c = small.tile([P, C], bf16)
                    nc.vector.tensor_scalar(
                        out=s2c[:], in0=p1["xa"][:, KNB, :],
                        scalar1=ef[:, 1:2], scalar2=None,
                        op0=mybir.AluOpType.mult,
                    )
                    diag15 = small.tile([P, P], bf16)
                    nc.scalar.mul(diag15[:], ident2_sb[:], ef[:, 0:1])
                    p1.update(inv=inv, s2c=s2c, diag15=diag15)

                if it < NTILES:
                    cur = st[it]
                    t = cur["t"]
                    u1 = tb.tile([P, J, C // 2], bf16)
                    nc.vector.tensor_tensor(
                        out=u1[:], in0=t[:, :, 0 : C // 2], in1=t[:, :, C // 2 : C],
                        op=mybir.AluOpType.add,
                    )
                    u2 = tb.tile([P, J, C // 4], bf16)
                    nc.vector.tensor_tensor(
                        out=u2[:], in0=u1[:, :, 0 : C // 4], in1=u1[:, :, C // 4 : C // 2],
                        op=mybir.AluOpType.add,
                    )
                    logit = small.tile([P, J], bf16)
                    with nc.allow_low_precision(reason="logit c-sum fits bf16"):
                        nc.vector.tensor_reduce(
                            out=logit[:], in_=u2[:],
                            axis=mybir.AxisListType.X, op=mybir.AluOpType.add,
                        )
                    e_sb = small.tile([P, JE], bf16)
                    sum_e = small.tile([P, 1], f32)
                    nc.scalar.activation(
                        out=e_sb[:, 0:J],
                        in_=logit[:],
                        func=mybir.ActivationFunctionType.Exp,
                        scale=SCALE,
                        accum_out=sum_e[:],
                    )
                    diag = diagp.tile([P, JSC, P], bf16)
                    nc.gpsimd.local_scatter(
                        out_ap=diag[:],
                        data_ap=e_sb[:, 0:16],
                        idxs_ap=sidx_sb[:],
                        channels=P,
                        num_elems=JSC * P,
                        num_idxs=16,
                    )
                    cur.update(e_sb=e_sb, sum_e=sum_e, diag=diag)

                if it - 1 in st:
                    # weighted-sum matmul block of tile i-1 (scatter finished
                    # last iteration; diag15/s2c just produced above)
                    p1 = st[it - 1]
                    s_ps = psS.tile([C, P], f32)
                    for j in range(JSC):
                        nc.tensor.matmul(
                            s_ps, lhsT=p1["xa"][:, j, :], rhs=p1["diag"][:, j, :],
                            start=(j == 0), stop=False,
                        )
                    nc.tensor.matmul(
                        s_ps, lhsT=p1["xa"][:, 15, :], rhs=p1["diag15"][:],
                        start=False, stop=False,
                    )
                    nc.tensor.matmul(
                        s_ps, lhsT=p1["s2c"][:], rhs=ident2_sb[:],
                        start=False, stop=True,
                    )
                    p1["s_ps"] = s_ps

                if it - 2 in st:
                    # output tail of tile i-2 (matmul block done last iteration)
                    p2 = st.pop(it - 2)
                    r2 = (it - 2) * P
                    s_sb = small.tile([C, P], bf16)
                    nc.scalar.copy(s_sb, p2["s_ps"])
                    o_ps = psA.tile([P, C], f32)
                    nc.tensor.matmul(
                        o_ps, lhsT=s_sb[:], rhs=wvt_sb[:], start=True, stop=True
                    )
                    o_sb = small.tile([P, C], bf16)
                    nc.scalar.mul(o_sb, o_ps, p2["inv"][:])
                    nc.gpsimd.dma_start(out=out[r2 : r2 + P, :], in_=o_sb[:])

    nc.compile()
    return nc


def _get_nc():
    if "nc" not in _cache:
        _cache["nc"] = _build()
    return _cache["nc"]


def _host_prep(fea_center, fea_near, wq, wk, wv):
    import ml_dtypes

    bf = ml_dtypes.bfloat16
    fea_center = np.asarray(fea_center, dtype=np.float32)
    fea_near = np.asarray(fea_near, dtype=np.float32)
    wq = np.asarray(wq, dtype=np.float32)
    wk = np.asarray(wk, dtype=np.float32)
    wv = np.asarray(wv, dtype=np.float32)

    amat = np.ascontiguousarray(wq.T @ wk).astype(bf)  # [c_center, c_near]
    wvt = np.ascontiguousarray(wv.T).astype(bf)  # [c_in, c_out]

    # [bs, n, 17, c]: near neighbors then the center as the 17th entry
    xfull = np.concatenate([fea_near, fea_center], axis=2).astype(bf)
    # transposed center features [bs, c, n]
    fcT = np.ascontiguousarray(np.transpose(fea_center[:, :, 0, :], (0, 2, 1))).astype(bf)

    ident2 = np.eye(P, dtype=np.float32).astype(bf)

    # local_scatter index table: partition p scatters e[p, j] to j*P + p
    pp = np.arange(P, dtype=np.int16)[:, None]
    sidx = np.full((P, 16), -1, dtype=np.int16)
    sidx[:, 0:JSC] = np.arange(JSC, dtype=np.int16)[None, :] * P + pp  # j = 0..14

    return xfull, fcT, amat, wvt, ident2, sidx


def kernel(fea_center, fea_near, wq, wk, wv):
    global last_exec_ns, last_results

    from concourse.bass_utils import run_bass_kernel_spmd

    xfull, fcT, amat, wvt, ident2, sidx = _host_prep(fea_center, fea_near, wq, wk, wv)

    nc = _get_nc()
    in_maps = []
    for b in range(BS):
        in_maps.append(
            {
                "xfull": np.ascontiguousarray(xfull[b]),
                "fcT": np.ascontiguousarray(fcT[b]),
                "amat": amat,
                "wvt": wvt,
                "ident2": ident2,
                "sidx": sidx,
            }
        )

    trace = bool(int(os.environ.get("BASS_KERNEL_TRACE", "0")))
    res = run_bass_kernel_spmd(nc, in_maps, core_ids=list(range(BS)), trace=trace)
    last_exec_ns = res.exec_time_ns
    last_results = res
    out = np.stack([res.results[b]["out"] for b in range(BS)], axis=0).astype(np.float32)
    return out
